# revision 6
# baseline (speedup 1.0000x reference)
"""Trainium2 Bass kernel for nn_AttentionBlockE3 (segment-softmax GNN attention).

Strategy (v3):
  * Nodes are bin-packed (LPT greedy on degree) into NCORES*CHUNKS bins of
    <=128 nodes with near-equal edge counts, so every (core, chunk) window
    has the same tile count T and the SPMD program is uniform with ~2% edge
    padding.
  * All numeric inputs are cast to bf16 on the host; cutoff/sqrt(60) is
    folded into q (exact algebra), so logits come straight from q'.k.
  * q/k are shipped transposed (feature dim on partitions, 3 full 128-dim
    blocks + one 96-dim block) so the per-head dot products run on the
    TENSOR engine: psum_w[e,h] += prod_a[:,e_block].T @ ones_a, where
    prod_a = q_a * k_a is a single bf16 vector multiply per block.
  * Softmax skips the max-subtraction entirely: |logit| <= ~6 for this
    data (cut in [0,1], q.k/sqrt(60) ~ N(0,1)), exp cannot overflow and
    the normalized weights are mathematically identical.
  * v is shipped edge-major with a 1.0 column appended per head ([h,61]
    interleaved); exp(w) runs on the scalar engine straight out of PSUM,
    is broadcast-expanded to 61 columns per head by a scalar-engine copy,
    and one flat bf16 vector multiply (2x mode) builds the weighted values
    plus the softmax denominator for 10 tiles at once; a one-hot(dst)
    matmul scatters both into PSUM [128 nodes, 488].
  * One-hot comes from a single-source tensor_scalar(is_equal) against a
    per-partition destination-slot scalar, all bf16.
"""
import numpy as np
from ml_dtypes import bfloat16

E, D, N, H = 200000, 480, 10000, 8
P = 128
NCORES = 8
CHUNKS = 10
NBINS = NCORES * CHUNKS
SCALE = 1.0 / np.sqrt(60.0)

# head-major column permutation: hm col h*60+d  ->  fused col PERM[h*60+d]
_BLOCK = [(0, 16), (128, 24), (320, 20)]


def _perm():
    cols = []
    for h in range(H):
        for off, hd in _BLOCK:
            cols.extend(range(off + h * hd, off + (h + 1) * hd))
    return np.array(cols, np.int64)


PERM = _perm()


def _plan_shard(dst):
    """Bin-pack nodes into NBINS bins (<=128 nodes, balanced edge counts).

    Returns dict with:
      T            tiles per chunk (uniform)
      eid          [NCORES, CHUNKS, T*128] edge id per slot (E = padding)
      dstrel       [NCORES, CHUNKS, T*128] dst slot in bin (-5.0 = padding)
      node_src     [N] row index into the concatenated device output
    """
    import heapq
    deg = np.bincount(dst, minlength=N)
    order = np.argsort(-deg, kind="stable")
    heap = [(0, b) for b in range(NBINS)]
    heapq.heapify(heap)
    bin_nodes = [[] for _ in range(NBINS)]
    for n in order:
        dn = int(deg[n])
        while True:
            load, b = heapq.heappop(heap)
            if len(bin_nodes[b]) < P:
                bin_nodes[b].append(n)
                heapq.heappush(heap, (load + dn, b))
                break
    bin_of = np.empty(N, np.int64)
    slot_of = np.empty(N, np.int64)
    for b, nodes in enumerate(bin_nodes):
        nodes = np.asarray(nodes, np.int64)
        bin_of[nodes] = b
        slot_of[nodes] = np.arange(len(nodes))
    ebin = bin_of[dst]
    eorder = np.argsort(ebin, kind="stable")
    counts = np.bincount(ebin, minlength=NBINS)
    T = int(np.ceil(counts.max() / P))
    if T % 2:
        T += 1                      # halves must tile evenly
    starts = np.zeros(NBINS + 1, np.int64)
    np.cumsum(counts, out=starts[1:])
    budget = T * P
    eid = np.full((NBINS, budget), E, np.int64)
    for b in range(NBINS):
        eid[b, :counts[b]] = eorder[starts[b]:starts[b + 1]]
    dstrel = np.full((NBINS, budget), -5.0, np.float32)
    valid = eid < E
    dstrel[valid] = slot_of[dst[eid[valid]]].astype(np.float32)
    # output row of node n: bin b -> core b//CHUNKS, chunk b%CHUNKS, slot
    node_src = (bin_of * P + slot_of).astype(np.int64)
    return {
        "T": T,
        "eid": eid.reshape(NCORES, CHUNKS, budget),
        "dstrel": dstrel.reshape(NCORES, CHUNKS, budget),
        "node_src": node_src,
    }


def _prep_global(key, value, query, cutoff):
    """Head-major, scaled, bf16, with a zero pad row at index E."""
    qs = query * (cutoff * SCALE)[:, None]
    qhm = np.zeros((E + 1, D), bfloat16)
    qhm[:E] = qs[:, PERM].astype(bfloat16)
    khm = np.zeros((E + 1, D), bfloat16)
    khm[:E] = key[:, PERM].astype(bfloat16)
    vhm = np.zeros((E + 1, H * 61), bfloat16)
    v61 = np.zeros((E, H, 61), np.float32)
    v61[:, :, :60] = value[:, PERM].reshape(E, H, 60)
    v61[:, :, 60] = 1.0
    vhm[:E] = v61.reshape(E, H * 61).astype(bfloat16)
    return qhm, khm, vhm


def _pack_core(core, plan, qhm, khm, vhm):
    T = plan["T"]
    HB = T * P // 2                              # edges per half-chunk
    eid = plan["eid"][core]                      # [CHUNKS, T*128]
    qg = qhm[eid]                                # [C, T*128, 480]
    kg = khm[eid]
    vg = vhm[eid]                                # [C, T*128, 488]
    C = CHUNKS

    def qk_trans(lo, hi, nblk):
        # -> [dims, C, 2, nblk, 2, HB] with (dim, c, half, blk, qk, e)
        dd = (hi - lo) // nblk
        qp = qg[:, :, lo:hi].reshape(C, 2, HB, nblk, dd)
        kp = kg[:, :, lo:hi].reshape(C, 2, HB, nblk, dd)
        st = np.stack([qp, kp], axis=4)          # [C, hf, e, blk, qk, dim]
        return np.ascontiguousarray(st.transpose(5, 0, 1, 3, 4, 2))

    qkt012 = qk_trans(0, 384, 3).reshape(P, C, 2, 3 * 2 * HB)
    qkt3 = qk_trans(384, 480, 1).reshape(96, C, 2, 2 * HB)
    vt = np.ascontiguousarray(
        vg.reshape(C, T, P, H * 61).transpose(2, 0, 1, 3)
    ).reshape(P, C, T * H * 61)
    dstr = np.ascontiguousarray(
        plan["dstrel"][core].reshape(C, T, P).transpose(2, 0, 1)
    ).reshape(P, C * T)
    ones = np.zeros((P, 4 * H), bfloat16)
    dims = np.arange(512)
    valid = dims < D
    ones[dims[valid] % P, (dims[valid] // P) * H + dims[valid] // 60] = 1
    return {"qkt012": qkt012, "qkt3": qkt3, "vt": vt, "dstr": dstr,
            "ones": ones}


def _build_program(T, reps=1):
    import contextlib

    import concourse.bacc as bacc
    import concourse.mybir as mybir
    import concourse.tile as tile

    f32 = mybir.dt.float32
    bf16 = mybir.dt.bfloat16
    C = CHUNKS
    EC = T * P                      # edges per chunk
    HB = EC // 2                    # edges per half-chunk
    TH = T // 2                     # tiles per half-chunk
    W61 = H * 61

    nc = bacc.Bacc("TRN2", target_bir_lowering=False, debug=False,
                   num_devices=NCORES)
    qkt012_d = nc.dram_tensor("qkt012", [P, C, 2, 3 * 2 * HB], bf16,
                              kind="ExternalInput").ap()
    qkt3_d = nc.dram_tensor("qkt3", [96, C, 2, 2 * HB], bf16,
                            kind="ExternalInput").ap()
    vt_d = nc.dram_tensor("vt", [P, C, T * W61], bf16,
                          kind="ExternalInput").ap()
    dstr_d = nc.dram_tensor("dstr", [P, C * T], f32,
                            kind="ExternalInput").ap()
    ones_d = nc.dram_tensor("ones", [P, 4 * H], bf16,
                            kind="ExternalInput").ap()
    out_d = nc.dram_tensor("out", [C * P, D], bf16,
                           kind="ExternalOutput").ap()

    with tile.TileContext(nc) as tc:
        with (
            tc.tile_pool(name="const", bufs=1) as const_pool,
            tc.tile_pool(name="qk012", bufs=3) as qk012_pool,
            tc.tile_pool(name="qk3", bufs=3) as qk3_pool,
            tc.tile_pool(name="vp", bufs=2) as v_pool,
            tc.tile_pool(name="prod", bufs=8) as prod_pool,
            tc.tile_pool(name="w", bufs=4) as w_pool,
            tc.tile_pool(name="wbig", bufs=3) as wbig_pool,
            tc.tile_pool(name="rhs", bufs=3) as rhs_pool,
            tc.tile_pool(name="oh", bufs=4) as oh_pool,
            tc.tile_pool(name="stat", bufs=4) as stat_pool,
            tc.tile_pool(name="outp", bufs=3) as out_pool,
            tc.tile_pool(name="psw", bufs=2, space="PSUM") as psw_pool,
            tc.tile_pool(name="pso", bufs=2, space="PSUM") as pso_pool,
        ):
            iota_i = const_pool.tile([P, P], mybir.dt.int32)
            nc.gpsimd.iota(iota_i[:], pattern=[[1, P]], base=0,
                           channel_multiplier=0)
            iota_f = const_pool.tile([P, P], f32)
            nc.vector.tensor_copy(iota_f[:], iota_i[:])
            ones_sb = const_pool.tile([P, 4 * H], bf16)
            nc.sync.dma_start(out=ones_sb[:], in_=ones_d[:, :])
            dstr_sb = const_pool.tile([P, C * T], f32)
            nc.sync.dma_start(out=dstr_sb[:], in_=dstr_d[:, :])

            def chunk_body(c):
                vt = v_pool.tile([P, T * W61], bf16)
                nc.sync.dma_start(out=vt[:], in_=vt_d[:, c, :])

                rhs_halves = []
                for hf in range(2):
                    qk012 = qk012_pool.tile([P, 3 * 2 * HB], bf16)
                    nc.sync.dma_start(out=qk012[:],
                                      in_=qkt012_d[:, c, hf, :])
                    qk3 = qk3_pool.tile([96, 2 * HB], bf16)
                    nc.sync.dma_start(out=qk3[:], in_=qkt3_d[:, c, hf, :])
                    prods = []
                    for a in range(3):
                        pr = prod_pool.tile([P, HB], bf16)
                        nc.vector.tensor_mul(
                            pr[:], qk012[:, a * 2 * HB:a * 2 * HB + HB],
                            qk012[:, a * 2 * HB + HB:(a + 1) * 2 * HB])
                        prods.append(pr)
                    pr3 = prod_pool.tile([96, HB], bf16)
                    nc.vector.tensor_mul(pr3[:], qk3[:, 0:HB], qk3[:, HB:])
                    prods.append(pr3)
                    psw = psw_pool.tile([P, TH * H], f32)
                    for tt in range(TH):
                        for a in range(4):
                            kdim = 96 if a == 3 else P
                            nc.tensor.matmul(
                                out=psw[:, tt * H:(tt + 1) * H],
                                lhsT=prods[a][:, tt * P:(tt + 1) * P],
                                rhs=ones_sb[0:kdim, a * H:(a + 1) * H],
                                start=(a == 0), stop=(a == 3))
                    wsb = w_pool.tile([P, TH * H], bf16)
                    nc.scalar.activation(wsb[:], psw[:],
                                         mybir.ActivationFunctionType.Exp)
                    wbig = wbig_pool.tile([P, TH * W61], bf16)
                    nc.scalar.activation(
                        wbig[:].rearrange("p (x d) -> p x d", d=61),
                        wsb[:].unsqueeze(2).to_broadcast([P, TH * H, 61]),
                        mybir.ActivationFunctionType.Copy)
                    rhs = rhs_pool.tile([P, TH * W61], bf16)
                    nc.vector.tensor_mul(
                        rhs[:], vt[:, hf * TH * W61:(hf + 1) * TH * W61],
                        wbig[:])
                    rhs_halves.append(rhs)

                pso = pso_pool.tile([P, W61], f32)
                for t in range(T):
                    hf, tt = divmod(t, TH)
                    g = c * T + t
                    oh = oh_pool.tile([P, P], bf16)
                    nc.vector.tensor_scalar(
                        out=oh[:], in0=iota_f[:],
                        scalar1=dstr_sb[:, g:g + 1], scalar2=None,
                        op0=mybir.AluOpType.is_equal)
                    nc.tensor.matmul(
                        out=pso[:], lhsT=oh[:],
                        rhs=rhs_halves[hf][:, tt * W61:(tt + 1) * W61],
                        start=(t == 0), stop=(t == T - 1))

                pv = pso[:].rearrange("p (h d) -> p h d", d=61)
                srec = stat_pool.tile([P, H, 1], f32)
                nc.vector.tensor_scalar_add(srec[:], pv[:, :, 60:61], 1e-16)
                nc.vector.reciprocal(srec[:], srec[:])
                outt = out_pool.tile([P, D], bf16)
                nc.vector.tensor_mul(
                    outt[:].rearrange("p (h d) -> p h d", h=H),
                    pv[:, :, 0:60],
                    srec[:].to_broadcast([P, H, 60]))
                nc.sync.dma_start(out=out_d[c * P:(c + 1) * P, :],
                                  in_=outt[:])

            loop = tc.For_i(0, reps, 1) if reps > 1 else contextlib.nullcontext()
            with loop:
                for c in range(CHUNKS):
                    chunk_body(c)

    nc.compile()
    return nc


def _unpack(plan, outs):
    """outs: list of per-core [C*128, 480] bf16 -> [N, 480] f32 fused."""
    allout = np.concatenate([np.asarray(o) for o in outs], axis=0)
    hm = allout[plan["node_src"]].astype(np.float32)    # [N, 480] head-major
    fused = np.empty((N, D), np.float32)
    fused[:, PERM] = hm
    return fused


def kernel(key, value, query, edge_weight_cutoff, edge_index, num_nodes):
    key = np.asarray(key, dtype=np.float32)
    value = np.asarray(value, dtype=np.float32)
    query = np.asarray(query, dtype=np.float32)
    cutoff = np.asarray(edge_weight_cutoff, dtype=np.float32)
    dst = np.asarray(edge_index)[1].astype(np.int64)

    plan = _plan_shard(dst)
    qhm, khm, vhm = _prep_global(key, value, query, cutoff)
    in_maps = [_pack_core(core, plan, qhm, khm, vhm)
               for core in range(NCORES)]

    nc = _build_program(plan["T"])

    from concourse.bass_utils import run_bass_kernel_spmd
    res = run_bass_kernel_spmd(nc, in_maps, core_ids=list(range(NCORES)))
    return np.ascontiguousarray(
        _unpack(plan, [r["out"] for r in res.results]))


if __name__ == "__main__":
    rng = np.random.default_rng(0)
    inputs = {
        "key": rng.standard_normal((E, D)).astype(np.float32),
        "value": rng.standard_normal((E, D)).astype(np.float32),
        "query": rng.standard_normal((E, D)).astype(np.float32),
        "edge_weight_cutoff": rng.random(E).astype(np.float32),
        "edge_index": rng.integers(0, N, (2, E)),
        "num_nodes": N,
    }
    out = kernel(**inputs)
    print("out", out.shape, out.dtype, float(np.abs(out).max()))


# revision 9
# speedup vs baseline: 1.1231x; 1.1231x over previous
"""Trainium2 Bass kernel for nn_AttentionBlockE3 (segment-softmax GNN attention).

Strategy (v3):
  * Nodes are bin-packed (LPT greedy on degree) into NCORES*CHUNKS bins of
    <=128 nodes with near-equal edge counts, so every (core, chunk) window
    has the same tile count T and the SPMD program is uniform with ~2% edge
    padding.
  * All numeric inputs are cast to bf16 on the host; cutoff/sqrt(60) is
    folded into q (exact algebra), so logits come straight from q'.k.
  * q/k are shipped transposed (feature dim on partitions, 3 full 128-dim
    blocks + one 96-dim block) so the per-head dot products run on the
    TENSOR engine: psum_w[e,h] += prod_a[:,e_block].T @ ones_a, where
    prod_a = q_a * k_a is a single bf16 vector multiply per block.
  * Softmax skips the max-subtraction entirely: |logit| <= ~6 for this
    data (cut in [0,1], q.k/sqrt(60) ~ N(0,1)), exp cannot overflow and
    the normalized weights are mathematically identical.
  * v is shipped edge-major with a 1.0 column appended per head ([h,61]
    interleaved); exp(w) runs on the scalar engine straight out of PSUM,
    is broadcast-expanded to 61 columns per head by a scalar-engine copy,
    and one flat bf16 vector multiply (2x mode) builds the weighted values
    plus the softmax denominator for 10 tiles at once; a one-hot(dst)
    matmul scatters both into PSUM [128 nodes, 488].
  * One-hot comes from a single-source tensor_scalar(is_equal) against a
    per-partition destination-slot scalar, all bf16.
"""
import numpy as np
from ml_dtypes import bfloat16

E, D, N, H = 200000, 480, 10000, 8
P = 128
NCORES = 8
CHUNKS = 10
NBINS = NCORES * CHUNKS
SCALE = 1.0 / np.sqrt(60.0)

# head-major column permutation: hm col h*60+d  ->  fused col PERM[h*60+d]
_BLOCK = [(0, 16), (128, 24), (320, 20)]


def _perm():
    cols = []
    for h in range(H):
        for off, hd in _BLOCK:
            cols.extend(range(off + h * hd, off + (h + 1) * hd))
    return np.array(cols, np.int64)


PERM = _perm()


def _plan_shard(dst):
    """Bin-pack nodes into NBINS bins (<=128 nodes, balanced edge counts).

    Returns dict with:
      T            tiles per chunk (uniform)
      eid          [NCORES, CHUNKS, T*128] edge id per slot (E = padding)
      dstrel       [NCORES, CHUNKS, T*128] dst slot in bin (-5.0 = padding)
      node_src     [N] row index into the concatenated device output
    """
    import heapq
    deg = np.bincount(dst, minlength=N)
    order = np.argsort(-deg, kind="stable")
    heap = [(0, b) for b in range(NBINS)]
    heapq.heapify(heap)
    bin_nodes = [[] for _ in range(NBINS)]
    for n in order:
        dn = int(deg[n])
        while True:
            load, b = heapq.heappop(heap)
            if len(bin_nodes[b]) < P:
                bin_nodes[b].append(n)
                heapq.heappush(heap, (load + dn, b))
                break
    bin_of = np.empty(N, np.int64)
    slot_of = np.empty(N, np.int64)
    for b, nodes in enumerate(bin_nodes):
        nodes = np.asarray(nodes, np.int64)
        bin_of[nodes] = b
        slot_of[nodes] = np.arange(len(nodes))
    ebin = bin_of[dst]
    eorder = np.argsort(ebin, kind="stable")
    counts = np.bincount(ebin, minlength=NBINS)
    T = int(np.ceil(counts.max() / P))
    if T % 2:
        T += 1                      # halves must tile evenly
    starts = np.zeros(NBINS + 1, np.int64)
    np.cumsum(counts, out=starts[1:])
    budget = T * P
    eid = np.full((NBINS, budget), E, np.int64)
    for b in range(NBINS):
        eid[b, :counts[b]] = eorder[starts[b]:starts[b + 1]]
    dstrel = np.full((NBINS, budget), -5.0, np.float32)
    valid = eid < E
    dstrel[valid] = slot_of[dst[eid[valid]]].astype(np.float32)
    # output row of node n: bin b -> core b//CHUNKS, chunk b%CHUNKS, slot
    node_src = (bin_of * P + slot_of).astype(np.int64)
    return {
        "T": T,
        "eid": eid.reshape(NCORES, CHUNKS, budget),
        "dstrel": dstrel.reshape(NCORES, CHUNKS, budget),
        "node_src": node_src,
    }


def _prep_global(key, value, query, cutoff):
    """Head-major, scaled, bf16, with a zero pad row at index E."""
    qs = query * (cutoff * SCALE)[:, None]
    qhm = np.zeros((E + 1, D), bfloat16)
    qhm[:E] = qs[:, PERM].astype(bfloat16)
    khm = np.zeros((E + 1, D), bfloat16)
    khm[:E] = key[:, PERM].astype(bfloat16)
    vhm = np.zeros((E + 1, H * 61), bfloat16)
    v61 = np.zeros((E, H, 61), np.float32)
    v61[:, :, :60] = value[:, PERM].reshape(E, H, 60)
    v61[:, :, 60] = 1.0
    vhm[:E] = v61.reshape(E, H * 61).astype(bfloat16)
    return qhm, khm, vhm


def _pack_core(core, plan, qhm, khm, vhm):
    T = plan["T"]
    HB = T * P // 2                              # edges per half-chunk
    eid = plan["eid"][core]                      # [CHUNKS, T*128]
    qg = qhm[eid]                                # [C, T*128, 480]
    kg = khm[eid]
    vg = vhm[eid]                                # [C, T*128, 488]
    C = CHUNKS

    def qk_trans(lo, hi, nblk):
        # -> [dims, C, nblk, 2, T*P] with (dim, c, blk, qk, e)
        dd = (hi - lo) // nblk
        qp = qg[:, :, lo:hi].reshape(C, T * P, nblk, dd)
        kp = kg[:, :, lo:hi].reshape(C, T * P, nblk, dd)
        st = np.stack([qp, kp], axis=3)          # [C, e, blk, qk, dim]
        return np.ascontiguousarray(st.transpose(4, 0, 2, 3, 1))

    qkt012 = qk_trans(0, 384, 3).reshape(P, C, 3 * 2 * T * P)
    qkt3 = qk_trans(384, 480, 1).reshape(96, C, 2 * T * P)
    vt = np.ascontiguousarray(
        vg.reshape(C, T, P, H * 61).transpose(2, 0, 1, 3)
    ).reshape(P, C, T * H * 61)
    dstr = np.ascontiguousarray(
        plan["dstrel"][core].reshape(C, T, P).transpose(2, 0, 1)
    ).reshape(P, C * T)
    ones = np.zeros((P, 4 * H), bfloat16)
    dims = np.arange(512)
    valid = dims < D
    ones[dims[valid] % P, (dims[valid] // P) * H + dims[valid] // 60] = 1
    return {"qkt012": qkt012, "qkt3": qkt3, "vt": vt, "dstr": dstr,
            "ones": ones}


def _build_program(T, reps=1, probe=None):
    import contextlib

    import concourse.bacc as bacc
    import concourse.mybir as mybir
    import concourse.tile as tile

    f32 = mybir.dt.float32
    bf16 = mybir.dt.bfloat16
    C = CHUNKS
    EC = T * P                      # edges per chunk
    HB = EC // 2                    # edges per half-chunk
    TH = T // 2                     # tiles per half-chunk
    W61 = H * 61

    nc = bacc.Bacc("TRN2", target_bir_lowering=False, debug=False,
                   num_devices=NCORES)
    qkt012_d = nc.dram_tensor("qkt012", [P, C, 3 * 2 * EC], bf16,
                              kind="ExternalInput").ap()
    qkt3_d = nc.dram_tensor("qkt3", [96, C, 2 * EC], bf16,
                            kind="ExternalInput").ap()
    vt_d = nc.dram_tensor("vt", [P, C, T * W61], bf16,
                          kind="ExternalInput").ap()
    dstr_d = nc.dram_tensor("dstr", [P, C * T], f32,
                            kind="ExternalInput").ap()
    ones_d = nc.dram_tensor("ones", [P, 4 * H], bf16,
                            kind="ExternalInput").ap()
    out_d = nc.dram_tensor("out", [C * P, D], bf16,
                           kind="ExternalOutput").ap()

    with tile.TileContext(nc) as tc:
        with (
            tc.tile_pool(name="const", bufs=1) as const_pool,
            tc.tile_pool(name="qk012", bufs=2) as qk012_pool,
            tc.tile_pool(name="qk3", bufs=2) as qk3_pool,
            tc.tile_pool(name="vp", bufs=2) as v_pool,
            tc.tile_pool(name="prod", bufs=6) as prod_pool,
            tc.tile_pool(name="w", bufs=4) as w_pool,
            tc.tile_pool(name="rhs", bufs=3) as rhs_pool,
            tc.tile_pool(name="oh", bufs=4) as oh_pool,
            tc.tile_pool(name="stat", bufs=4) as stat_pool,
            tc.tile_pool(name="outp", bufs=3) as out_pool,
            tc.tile_pool(name="psw", bufs=2, space="PSUM") as psw_pool,
            tc.tile_pool(name="pso", bufs=2, space="PSUM") as pso_pool,
        ):
            iota_i = const_pool.tile([P, P], mybir.dt.int32)
            nc.gpsimd.iota(iota_i[:], pattern=[[1, P]], base=0,
                           channel_multiplier=0)
            iota_f = const_pool.tile([P, P], f32)
            nc.vector.tensor_copy(iota_f[:], iota_i[:])
            ones_sb = const_pool.tile([P, 4 * H], bf16)
            nc.sync.dma_start(out=ones_sb[:], in_=ones_d[:, :])
            dstr_sb = const_pool.tile([P, C * T], f32)
            nc.sync.dma_start(out=dstr_sb[:], in_=dstr_d[:, :])

            def chunk_body(c):
                vt = v_pool.tile([P, T * W61], bf16)
                nc.sync.dma_start(out=vt[:], in_=vt_d[:, c, :])
                qk012 = qk012_pool.tile([P, 3 * 2 * EC], bf16)
                nc.sync.dma_start(out=qk012[:], in_=qkt012_d[:, c, :])
                qk3 = qk3_pool.tile([96, 2 * EC], bf16)
                nc.sync.dma_start(out=qk3[:], in_=qkt3_d[:, c, :])

                rhs_halves = []
                for hf in range(2):
                    if probe == "dmafloor":
                        prods = [qk012[:, (a * 2 + 1) * EC + hf * HB:
                                       (a * 2 + 1) * EC + (hf + 1) * HB]
                                 for a in range(3)]
                        prods.append(qk3[:, EC + hf * HB:EC + (hf + 1) * HB])
                    else:
                        prods = []
                        for a in range(3):
                            pr = prod_pool.tile([P, HB], bf16)
                            base = a * 2 * EC
                            nc.vector.tensor_mul(
                                pr[:],
                                qk012[:, base + hf * HB:base + (hf + 1) * HB],
                                qk012[:, base + EC + hf * HB:
                                      base + EC + (hf + 1) * HB])
                            prods.append(pr)
                        pr3 = prod_pool.tile([96, HB], bf16)
                        nc.vector.tensor_mul(
                            pr3[:], qk3[:, hf * HB:(hf + 1) * HB],
                            qk3[:, EC + hf * HB:EC + (hf + 1) * HB])
                        prods.append(pr3)
                    psw = psw_pool.tile([P, TH * H], f32)
                    for tt in range(TH):
                        for a in range(4):
                            kdim = 96 if a == 3 else P
                            lhs = prods[a]
                            lhs = lhs[:, tt * P:(tt + 1) * P] \
                                if probe is None else lhs[0:kdim,
                                                          tt * P:(tt + 1) * P]
                            nc.tensor.matmul(
                                out=psw[:, tt * H:(tt + 1) * H],
                                lhsT=lhs,
                                rhs=ones_sb[0:kdim, a * H:(a + 1) * H],
                                start=(a == 0), stop=(a == 3))
                    wsb = w_pool.tile([P, TH * H], bf16)
                    nc.scalar.activation(wsb[:], psw[:],
                                         mybir.ActivationFunctionType.Exp)
                    if probe == "dmafloor":
                        rhs_halves.append(
                            vt[:, hf * TH * W61:(hf + 1) * TH * W61])
                    else:
                        rhs = rhs_pool.tile([P, TH * W61], bf16)
                        nc.vector.tensor_mul(
                            rhs[:].rearrange("p (x d) -> p x d", d=61),
                            vt[:, hf * TH * W61:(hf + 1) * TH * W61].rearrange(
                                "p (x d) -> p x d", d=61),
                            wsb[:].unsqueeze(2).to_broadcast([P, TH * H, 61]))
                        rhs_halves.append(rhs)

                pso = pso_pool.tile([P, W61], f32)
                for t in range(T):
                    hf, tt = divmod(t, TH)
                    g = 0 if probe == "dmafloor" else c * T + t
                    oh = oh_pool.tile([P, P], bf16)
                    nc.vector.tensor_scalar(
                        out=oh[:], in0=iota_f[:],
                        scalar1=dstr_sb[:, g:g + 1], scalar2=None,
                        op0=mybir.AluOpType.is_equal)
                    nc.tensor.matmul(
                        out=pso[:], lhsT=oh[:],
                        rhs=rhs_halves[hf][:, tt * W61:(tt + 1) * W61],
                        start=(t == 0), stop=(t == T - 1))

                pv = pso[:].rearrange("p (h d) -> p h d", d=61)
                srec = stat_pool.tile([P, H, 1], f32)
                nc.vector.tensor_scalar_add(srec[:], pv[:, :, 60:61], 1e-16)
                nc.vector.reciprocal(srec[:], srec[:])
                outt = out_pool.tile([P, D], bf16)
                nc.vector.tensor_mul(
                    outt[:].rearrange("p (h d) -> p h d", h=H),
                    pv[:, :, 0:60],
                    srec[:].to_broadcast([P, H, 60]))
                nc.sync.dma_start(out=out_d[c * P:(c + 1) * P, :],
                                  in_=outt[:])

            loop = tc.For_i(0, reps, 1) if reps > 1 else contextlib.nullcontext()
            with loop:
                for c in range(CHUNKS):
                    chunk_body(c)

    nc.compile()
    return nc


def _unpack(plan, outs):
    """outs: list of per-core [C*128, 480] bf16 -> [N, 480] f32 fused."""
    allout = np.concatenate([np.asarray(o) for o in outs], axis=0)
    hm = allout[plan["node_src"]].astype(np.float32)    # [N, 480] head-major
    fused = np.empty((N, D), np.float32)
    fused[:, PERM] = hm
    return fused


def kernel(key, value, query, edge_weight_cutoff, edge_index, num_nodes):
    key = np.asarray(key, dtype=np.float32)
    value = np.asarray(value, dtype=np.float32)
    query = np.asarray(query, dtype=np.float32)
    cutoff = np.asarray(edge_weight_cutoff, dtype=np.float32)
    dst = np.asarray(edge_index)[1].astype(np.int64)

    plan = _plan_shard(dst)
    qhm, khm, vhm = _prep_global(key, value, query, cutoff)
    in_maps = [_pack_core(core, plan, qhm, khm, vhm)
               for core in range(NCORES)]

    nc = _build_program(plan["T"])

    from concourse.bass_utils import run_bass_kernel_spmd
    res = run_bass_kernel_spmd(nc, in_maps, core_ids=list(range(NCORES)))
    return np.ascontiguousarray(
        _unpack(plan, [r["out"] for r in res.results]))


if __name__ == "__main__":
    rng = np.random.default_rng(0)
    inputs = {
        "key": rng.standard_normal((E, D)).astype(np.float32),
        "value": rng.standard_normal((E, D)).astype(np.float32),
        "query": rng.standard_normal((E, D)).astype(np.float32),
        "edge_weight_cutoff": rng.random(E).astype(np.float32),
        "edge_index": rng.integers(0, N, (2, E)),
        "num_nodes": N,
    }
    out = kernel(**inputs)
    print("out", out.shape, out.dtype, float(np.abs(out).max()))
